# revision 13
# baseline (speedup 1.0000x reference)
"""Trainium2 Bass kernel for nn_HardestContrastiveLoss.

Strategy (1D row-parallel cdist, per sharding hint):
  - Host: gather the selected correspondences (pure indexing/layout), build
    transposed operand blocks, shard 8192 selected rows as 1024 rows/core.
  - Device (per core, identical program, different data):
      * scale gathered src feats by -2, rigid-transform gathered src points
        (rot/trans via a small matmul + fused add/scale), square + ones-matmul
        for the |.|^2 rows -> extended matmul operands
      * feats distance^2 matrix [1024, 8192] and pts distance^2 matrix as
        float32r matmuls with extended vectors [-2a, 1, |a|^2] . [b, |b|^2, 1]
        (f32r runs the PE at 1 cycle/row vs 4 for plain f32)
      * masks from pts-d2 on ScalarE: t = relu(S*(c - pd2)) in {0, >=1e3}
      * select+reduce on VectorE:
          pos: rowmax over min(fd2, t1)   (t1 huge iff pos pair)
          neg: rowmin over max(fd2, t2)   (t2 huge iff NOT a neg pair)
        all in squared-distance space (sqrt deferred to [128, 8] tail)
      * tail: sqrt, relu thresholds, partition-sum via ones-matmul
  - Host: sum the 8 per-core [2,1] partials, divide by N (the "all-reduce").

The [8192, 8192] distance matrices never leave PSUM.
"""

import numpy as np

N_SEL = 8192
N_CORES = 8
ROWS_PER_CORE = N_SEL // N_CORES  # 1024
M_TILES = ROWS_PER_CORE // 128  # 8
NT = 512
N_TILES = N_SEL // NT  # 16
K_EXT = 69  # feats ext block at partitions 0:34, pts ext block at 64:69

EPS = 1e-7
POS_RADIUS = 0.0375
NEG_RADIUS = 0.1
POS_THRESH = 0.1
NEG_THRESH = 1.4
C1 = float(np.float32(POS_RADIUS**2 - EPS))  # pos: pd2 < C1
C2 = float(np.float32(NEG_RADIUS**2 - EPS))  # neg: pd2 > C2
S = 1.0e13
BIG = 100000.0

_PROGRAM_CACHE: dict = {}
KERNEL_CFG = {"mm": "f32r", "seldt": "bf16"}


def build_program(repeat: int = 1, mm: str = "f32r",
                  seldt: str = "f32", **_ignored):
    """Build the Bass program (one NeuronCore, run SPMD on 8)."""
    import concourse.bacc as bacc
    import concourse.mybir as mybir
    import concourse.tile as tile

    f32 = mybir.dt.float32
    f32r = mybir.dt.float32r if mm == "f32r" else mybir.dt.float32
    seldt = mybir.dt.bfloat16 if seldt == "bf16" else mybir.dt.float32
    A = mybir.AluOpType
    AF = mybir.ActivationFunctionType
    X = mybir.AxisListType.X

    nc = bacc.Bacc("TRN2", target_bir_lowering=False, debug=False,
                   num_devices=N_CORES)
    srcT_d = nc.dram_tensor("srcT", [K_EXT, ROWS_PER_CORE], f32,
                            kind="ExternalInput").ap()
    tgtT_d = nc.dram_tensor("tgtT", [K_EXT, N_SEL], f32,
                            kind="ExternalInput").ap()
    rtt_d = nc.dram_tensor("rtt", [3, 4], f32, kind="ExternalInput").ap()
    out_d = nc.dram_tensor("out", [2, 1], f32, kind="ExternalOutput").ap()

    RCH = 2048  # rhs prep chunk (DMA/square/norm pipeline granularity)

    with tile.TileContext(nc) as tc:
        with (
            tc.tile_pool(name="big", bufs=1) as big,
            tc.tile_pool(name="mask", bufs=4) as mask_p,
            tc.tile_pool(name="val", bufs=4) as val_p,
            tc.tile_pool(name="red", bufs=2) as red_p,
            tc.tile_pool(name="small", bufs=4) as small,
            tc.tile_pool(name="pf", bufs=2, space="PSUM") as pf_p,
            tc.tile_pool(name="pp", bufs=2, space="PSUM") as pp_p,
        ):
            rhs = big.tile([K_EXT, N_SEL], f32r, tag="rhs")
            lhs = big.tile([K_EXT, ROWS_PER_CORE], f32r, tag="lhs")
            rtt_sb = big.tile([3, 4], f32r, tag="rtt")
            praw = big.tile([3, ROWS_PER_CORE], f32r, tag="praw")
            prot = big.tile([3, ROWS_PER_CORE], f32r, tag="prot")
            sqt = big.tile([K_EXT, N_SEL], f32r, tag="sqt")
            sqs = big.tile([K_EXT, ROWS_PER_CORE], f32r, tag="sqs")
            nlT = big.tile([K_EXT, 2], f32, tag="nlT")
            nlS = big.tile([K_EXT, 2], f32, tag="nlS")
            nlTr = big.tile([K_EXT, 2], f32r, tag="nlTr")
            nlSr = big.tile([K_EXT, 2], f32r, tag="nlSr")
            nrmT = big.tile([2, N_SEL], f32r, tag="nrmT")
            nrmS = big.tile([2, ROWS_PER_CORE], f32r, tag="nrmS")
            ones128 = big.tile([128, 1], f32, tag="ones128")
            fp2all = big.tile([128, M_TILES], f32, tag="fp2all")
            cn2all = big.tile([128, M_TILES], f32, tag="cn2all")
            accT = big.tile([128, 2], f32, tag="accT")
            b1 = big.tile([128, 1], f32, tag="b1")
            b2 = big.tile([128, 1], f32, tag="b2")
            beps = big.tile([128, 1], f32, tag="beps")
            bpos = big.tile([128, 1], f32, tag="bpos")
            bneg = big.tile([128, 1], f32, tag="bneg")

            # rhs DMA in column chunks so squares/norms pipeline behind it
            for ch in range(N_SEL // RCH):
                sl = slice(ch * RCH, (ch + 1) * RCH)
                nc.sync.dma_start(rhs[:, sl], tgtT_d[:, sl].bitcast(f32r))
            nc.sync.dma_start(lhs[:], srcT_d[:].bitcast(f32r))
            nc.sync.dma_start(rtt_sb[:], rtt_d[:].bitcast(f32r))
            nc.gpsimd.memset(ones128[:], 1.0)
            nc.gpsimd.memset(b1[:], float(np.float32(S * C1)))
            nc.gpsimd.memset(b2[:], float(np.float32(S * C2)))
            nc.gpsimd.memset(beps[:], EPS)
            nc.gpsimd.memset(bpos[:], -POS_THRESH)
            nc.gpsimd.memset(bneg[:], NEG_THRESH)
            nc.sync.dma_start(praw[:], srcT_d[64:67, :].bitcast(f32r))
            nc.gpsimd.memset(nlT[:], 0.0)
            nc.gpsimd.memset(nlT[0:32, 0:1], 1.0)
            nc.gpsimd.memset(nlT[64:67, 1:2], 1.0)
            nc.gpsimd.memset(nlS[:], 0.0)
            nc.gpsimd.memset(nlS[0:32, 0:1], 0.25)
            nc.gpsimd.memset(nlS[64:67, 1:2], 0.25)
            nc.scalar.copy(nlTr[:], nlT[:])
            nc.scalar.copy(nlSr[:], nlS[:])

            # ---- src-side prep: lhs rows [-2sf |1| nsf | -2sp' |1| nsp'] ----
            nc.scalar.activation(lhs[0:32, :], lhs[0:32, :], AF.Copy,
                                 bias=0.0, scale=-2.0)
            for ch in range(ROWS_PER_CORE // NT):
                sl = slice(ch * NT, (ch + 1) * NT)
                psr = pf_p.tile([3, NT], f32, tag="psf")
                nc.tensor.matmul(out=psr[:], lhsT=rtt_sb[0:3, 0:3],
                                 rhs=praw[:, sl],
                                 start=True, stop=True)
                # -2 * (R p + t)
                nc.vector.tensor_scalar(
                    out=prot[:, sl], in0=psr[:],
                    scalar1=rtt_sb[0:3, 3:4].bitcast(f32), scalar2=-2.0,
                    op0=A.add, op1=A.mult)
            # move rotated pts into the pts ext block (partition shift -> DMA)
            nc.sync.dma_start(lhs[64:67, :], prot[:])
            nc.scalar.activation(sqs[:], lhs[0:K_EXT, :], AF.Square)
            for ch in range(ROWS_PER_CORE // NT):
                sl = slice(ch * NT, (ch + 1) * NT)
                psn = pf_p.tile([2, NT], f32, tag="psf")
                nc.tensor.matmul(out=psn[:], lhsT=nlSr[:],
                                 rhs=sqs[:, sl],
                                 start=True, stop=True)
                nc.scalar.copy(nrmS[:, sl], psn[:])
            nc.sync.dma_start(lhs[33:34, :], nrmS[0:1, :])
            nc.sync.dma_start(lhs[68:69, :], nrmS[1:2, :])

            # ---- tgt-side prep: rhs rows [tf | ntf |1| tp | ntp |1] ----
            for ch in range(N_SEL // RCH):
                sl = slice(ch * RCH, (ch + 1) * RCH)
                nc.scalar.activation(sqt[:, sl], rhs[0:K_EXT, sl], AF.Square)
            for ch in range(N_TILES):
                sl = slice(ch * NT, (ch + 1) * NT)
                psn = pf_p.tile([2, NT], f32, tag="psf")
                nc.tensor.matmul(out=psn[:], lhsT=nlTr[:],
                                 rhs=sqt[:, sl],
                                 start=True, stop=True)
                nc.scalar.copy(nrmT[:, sl], psn[:])
            nc.sync.dma_start(rhs[32:33, :], nrmT[0:1, :])
            nc.sync.dma_start(rhs[67:68, :], nrmT[1:2, :])

            GNT = 2 * NT  # 1024-wide post-processing (two PSUM banks)
            NP = N_SEL // GNT  # 8 n-pairs

            # t1 == relu(t2 - (b2-b1)) exactly: derive t1 on DVE (2x bf16)
            # for alternate pairs to balance ACT vs DVE load.
            DELTA = float(np.float32(S * C2) - np.float32(S * C1))

            acc_ps = [big.tile([128, GNT], seldt, tag=f"acc_p{m}",
                                name=f"acc_p{m}") for m in range(M_TILES)]
            acc_ns = [big.tile([128, GNT], seldt, tag=f"acc_n{m}",
                                name=f"acc_n{m}") for m in range(M_TILES)]

            def main_loop(_iv=None):
                for m in range(M_TILES):
                    msl = slice(m * 128, (m + 1) * 128)
                    acc_p = acc_ps[m]
                    acc_n = acc_ns[m]
                    for n in range(NP):
                        psf = pf_p.tile([128, GNT], f32, tag="psf")
                        psp = pp_p.tile([128, GNT], f32, tag="psp")
                        for g in range(2):
                            nsl = slice(n * GNT + g * NT,
                                        n * GNT + (g + 1) * NT)
                            gsl = slice(g * NT, (g + 1) * NT)
                            nc.tensor.matmul(out=psf[:, gsl],
                                             lhsT=lhs[0:34, msl],
                                             rhs=rhs[0:34, nsl],
                                             start=True, stop=True)
                            nc.tensor.matmul(out=psp[:, gsl],
                                             lhsT=lhs[64:69, msl],
                                             rhs=rhs[64:69, nsl],
                                             start=True, stop=True)
                        psfB = val_p.tile([128, GNT], seldt, tag="psfB")
                        nc.scalar.copy(psfB[:], psf[:])
                        t2 = mask_p.tile([128, GNT], seldt, tag="t2")
                        nc.scalar.activation(t2[:], psp[:], AF.Relu,
                                             bias=b2[:], scale=-S)
                        t1 = mask_p.tile([128, GNT], seldt, tag="t1")
                        if n % 2 == 1:
                            nc.vector.tensor_scalar(
                                out=t1[:], in0=t2[:], scalar1=DELTA,
                                scalar2=0.0, op0=A.subtract, op1=A.max)
                        else:
                            nc.scalar.activation(t1[:], psp[:], AF.Relu,
                                                 bias=b1[:], scale=-S)
                        if n == 0:
                            nc.vector.tensor_tensor(out=acc_p[:], in0=psfB[:],
                                                    in1=t1[:], op=A.min)
                            nc.vector.tensor_tensor(out=acc_n[:], in0=psfB[:],
                                                    in1=t2[:], op=A.max)
                        else:
                            sc1 = val_p.tile([128, GNT], seldt, tag="sc1")
                            nc.vector.tensor_tensor(out=sc1[:], in0=psfB[:],
                                                    in1=t1[:], op=A.min)
                            sc2 = val_p.tile([128, GNT], seldt, tag="sc2")
                            nc.vector.tensor_tensor(out=sc2[:], in0=psfB[:],
                                                    in1=t2[:], op=A.max)
                            nc.vector.tensor_tensor(out=acc_p[:], in0=acc_p[:],
                                                    in1=sc1[:], op=A.max)
                            nc.vector.tensor_tensor(out=acc_n[:], in0=acc_n[:],
                                                    in1=sc2[:], op=A.min)

            if repeat == 1:
                main_loop()
            else:
                with tc.For_i(0, repeat, 1) as iv:
                    main_loop(iv)

            # ---- tail: per-m reductions, then sqrt / relu / partition sums ----
            for m in range(M_TILES):
                nc.vector.tensor_reduce(out=fp2all[:, m:m + 1],
                                        in_=acc_ps[m][:], op=A.max, axis=X)
                nc.vector.tensor_reduce(out=cn2all[:, m:m + 1],
                                        in_=acc_ns[m][:], op=A.min, axis=X)
            fp = small.tile([128, M_TILES], f32, tag="fp")
            cn = small.tile([128, M_TILES], f32, tag="cn")
            nc.scalar.activation(fp[:], fp2all[:], AF.Sqrt, bias=beps[:])
            nc.scalar.activation(cn[:], cn2all[:], AF.Sqrt, bias=beps[:])
            pl = small.tile([128, M_TILES], f32, tag="pl")
            nl = small.tile([128, M_TILES], f32, tag="nl")
            nc.scalar.activation(pl[:], fp[:], AF.Relu, bias=bpos[:])
            nc.scalar.activation(nl[:], cn[:], AF.Relu, bias=bneg[:],
                                 scale=-1.0)
            nc.vector.tensor_reduce(out=accT[:, 0:1], in_=pl[:], op=A.add,
                                    axis=X)
            nc.vector.tensor_reduce(out=accT[:, 1:2], in_=nl[:], op=A.add,
                                    axis=X)
            pso = pf_p.tile([2, 1], f32, tag="psf")
            nc.tensor.matmul(out=pso[:], lhsT=accT[:], rhs=ones128[:],
                             start=True, stop=True)
            res_sb = small.tile([2, 1], f32, tag="res")
            nc.scalar.copy(res_sb[:], pso[:])
            nc.sync.dma_start(out_d[:], res_sb[:])

    nc.compile()
    return nc


def make_in_maps(src_pcd, tgt_pcd, src_feats, tgt_feats, correspondence,
                 rot, trans):
    """Host-side gather/shard/layout (indexing + transpose only)."""
    ci = np.asarray(correspondence[:, 0]).astype(np.int64)
    cj = np.asarray(correspondence[:, 1]).astype(np.int64)
    src_pcd = np.asarray(src_pcd, np.float32)
    tgt_pcd = np.asarray(tgt_pcd, np.float32)
    src_feats = np.asarray(src_feats, np.float32)
    tgt_feats = np.asarray(tgt_feats, np.float32)

    tgtT = np.zeros((K_EXT, N_SEL), np.float32)
    tgtT[0:32] = tgt_feats[cj].T
    tgtT[33] = 1.0
    tgtT[64:67] = tgt_pcd[cj].T
    tgtT[68] = 1.0

    srcT = np.zeros((K_EXT, N_SEL), np.float32)
    srcT[0:32] = src_feats[ci].T  # device scales by -2
    srcT[32] = 1.0
    srcT[64:67] = src_pcd[ci].T  # device applies rot/trans and -2
    srcT[67] = 1.0

    rtt = np.zeros((3, 4), np.float32)
    rtt[:, 0:3] = np.asarray(rot, np.float32).T
    rtt[:, 3] = np.asarray(trans, np.float32)[:, 0]

    in_maps = []
    for c in range(N_CORES):
        sl = slice(c * ROWS_PER_CORE, (c + 1) * ROWS_PER_CORE)
        in_maps.append({
            "srcT": np.ascontiguousarray(srcT[:, sl]),
            "tgtT": tgtT,
            "rtt": rtt,
        })
    return in_maps


def combine_outputs(results):
    """Host-side unshard: sum per-core partial sums, divide by N."""
    tot = np.zeros(2, np.float32)
    for r in results:
        tot += r["out"][:, 0].astype(np.float32)
    loss = np.float32(tot[0] / np.float32(N_SEL) + tot[1] / np.float32(N_SEL))
    return np.float32(loss)


def kernel(src_pcd, tgt_pcd, src_feats, tgt_feats, correspondence, rot,
           trans):
    from concourse import bass_utils

    key = ("prog", 1, KERNEL_CFG["mm"], KERNEL_CFG["seldt"])
    if key not in _PROGRAM_CACHE:
        _PROGRAM_CACHE[key] = build_program(repeat=1, mm=KERNEL_CFG["mm"],
                                            seldt=KERNEL_CFG["seldt"])
    nc = _PROGRAM_CACHE[key]
    in_maps = make_in_maps(src_pcd, tgt_pcd, src_feats, tgt_feats,
                           correspondence, rot, trans)
    res = bass_utils.run_bass_kernel_spmd(nc, in_maps,
                                          core_ids=list(range(N_CORES)))
    return combine_outputs(res.results)
